# revision 6
# baseline (speedup 1.0000x reference)
"""Causal self-attention (B=2, S=2048, D=2048, H=16, HD=128) on 8 TRN2 cores.

Sharding: core c -> batch b = c//4, heads 4*(c%4)..4*(c%4)+3 (tensor-parallel
over heads within a batch; data-parallel over batch across core groups).
Each core:
  phase 1: Q^T/K^T (RoPE applied) and V projections for its 4 heads, streamed
           over s-blocks of 512, bounced to DRAM scratch.
  phase 2: causal attention per head in transposed-score layout
           (S^T[k,q] tiles), softmax without max-subtraction, row-sums via a
           ones-vector matmul, PV produces ctx^T[hd,q] directly.
  phase 3: partial output projection ctx^T.T @ Wo_rows -> [2048, 2048].
Host sums the 4 partials per batch.

All matmul operands are bitcast to float32r (FP22 multiply, FP32 accumulate)
for full PE rate; everything else is fp32.
"""

import math

import numpy as np

import concourse.bacc as bacc
import concourse.mybir as mybir
from concourse.tile import TileContext
from concourse.bass_utils import run_bass_kernel_spmd

B, S, D = 2, 2048, 2048
H, HD = 16, 128
ROPE_THETA = 10000.0

N_CORES = 8
CORES_PER_BATCH = 4
HPC = H // (N_CORES // B)  # heads per core = 4
HL = HPC * HD              # 512 local head-dim columns
NDC = D // 128             # 16 contraction chunks
NSB = S // 512             # 4 s-blocks
NKC = S // 128             # 16 k-chunks

F32 = mybir.dt.float32
F32R = mybir.dt.float32r
AF = mybir.ActivationFunctionType

USE_F32R = True
# dtype for every tensor that feeds the PE: FP22 multiply at full rate
FMM = F32R if USE_F32R else F32


def _mm(nc, out, lhsT, rhs, start, stop):
    nc.tensor.matmul(out, lhsT, rhs, start=start, stop=stop)


def _build():
    nc = bacc.Bacc("TRN2", target_bir_lowering=False, debug=False)

    xT = nc.dram_tensor("xT", [D, S], FMM, kind="ExternalInput")
    wq = nc.dram_tensor("wq", [D, HL], FMM, kind="ExternalInput")
    wk = nc.dram_tensor("wk", [D, HL], FMM, kind="ExternalInput")
    wv = nc.dram_tensor("wv", [D, HL], FMM, kind="ExternalInput")
    wo = nc.dram_tensor("wo", [HL, D], FMM, kind="ExternalInput")
    cosT = nc.dram_tensor("cosT", [HD, S], F32, kind="ExternalInput")
    sinT = nc.dram_tensor("sinT", [HD, S], F32, kind="ExternalInput")
    pmatT = nc.dram_tensor("pmatT", [HD, HD], FMM, kind="ExternalInput")
    maskT = nc.dram_tensor("maskT", [128, 512], F32, kind="ExternalInput")
    onesd = nc.dram_tensor("onesd", [128, 128], FMM, kind="ExternalInput")
    out = nc.dram_tensor("out", [S, D], F32, kind="ExternalOutput")

    with TileContext(nc) as tc:
        with (
            tc.tile_pool(name="dram", bufs=1, space="DRAM") as dpool,
            tc.tile_pool(name="consts", bufs=1) as consts,
            tc.tile_pool(name="psA", bufs=3, space="PSUM") as psA,
            tc.tile_pool(name="psB", bufs=2, space="PSUM") as psB,
            tc.tile_pool(name="psC", bufs=2, space="PSUM") as psC,
        ):
            qTs = dpool.tile([HPC, HD, S], FMM, name="qTs")
            kTs = dpool.tile([HPC, HD, S], FMM, name="kTs")
            vs = dpool.tile([S, HL], FMM, name="vs")

            cos_sb = consts.tile([HD, S], F32, name="cos_sb")
            nc.sync.dma_start(out=cos_sb[:], in_=cosT[:])
            sin_sb = consts.tile([HD, S], F32, name="sin_sb")
            nc.sync.dma_start(out=sin_sb[:], in_=sinT[:])
            pmat_sb = consts.tile([HD, HD], FMM, name="pmat_sb")
            nc.sync.dma_start(out=pmat_sb[:], in_=pmatT[:])
            mask_sb = consts.tile([128, 512], F32, name="mask_sb")
            nc.sync.dma_start(out=mask_sb[:], in_=maskT[:])
            ones_sb = consts.tile([128, 128], FMM, name="ones_sb")
            nc.sync.dma_start(out=ones_sb[:], in_=onesd[:])
            ones_col = ones_sb[:, 0:1]
            ones_row = ones_sb[0:1, :]

            # ---------------- phase 1: projections + RoPE ----------------
            with (
                tc.tile_pool(name="wpool", bufs=1) as wpool,
                tc.tile_pool(name="xtp", bufs=17) as xtp,
                tc.tile_pool(name="st1", bufs=3) as st1,
            ):
                wq_sb = wpool.tile([128, NDC * HL], FMM, name="wq_sb")
                wk_sb = wpool.tile([128, NDC * HL], FMM, name="wk_sb")
                wv_sb = wpool.tile([128, NDC * HL], FMM, name="wv_sb")
                for w_sb, w_d in ((wq_sb, wq), (wk_sb, wk), (wv_sb, wv)):
                    for dc in range(NDC):
                        nc.sync.dma_start(
                            out=w_sb[:, dc * HL:(dc + 1) * HL],
                            in_=w_d[dc * 128:(dc + 1) * 128, :],
                        )

                for sb in range(NSB):
                    sl = slice(sb * 512, (sb + 1) * 512)
                    xts = []
                    for dc in range(NDC):
                        xt = xtp.tile([128, 512], FMM, tag="xt")
                        nc.sync.dma_start(out=xt[:], in_=xT[dc * 128:(dc + 1) * 128, sl])
                        xts.append(xt)

                    # Q^T and K^T per head, with RoPE
                    for w_sb, dst in ((wq_sb, qTs), (wk_sb, kTs)):
                        for h in range(HPC):
                            ps = psA.tile([128, 512], F32, tag="a")
                            for dc in range(NDC):
                                _mm(nc, ps[:],
                                    w_sb[:, dc * HL + h * HD: dc * HL + (h + 1) * HD],
                                    xts[dc][:],
                                    start=(dc == 0), stop=(dc == NDC - 1))
                            qraw = st1.tile([128, 512], FMM, tag="qraw")
                            nc.scalar.activation(qraw[:], ps[:], AF.Copy)
                            rot = psB.tile([128, 512], F32, tag="b")
                            _mm(nc, rot[:], pmat_sb[:], qraw[:], start=True, stop=True)
                            acos = st1.tile([128, 512], F32, tag="acos")
                            nc.vector.tensor_mul(acos[:], ps[:], cos_sb[:, sl])
                            rsin = st1.tile([128, 512], F32, tag="rsin")
                            nc.vector.tensor_mul(rsin[:], rot[:], sin_sb[:, sl])
                            rope = st1.tile([128, 512], FMM, tag="rope")
                            nc.vector.tensor_add(rope[:], rsin[:], acos[:])
                            nc.sync.dma_start(out=dst[h, :, sl], in_=rope[:])

                    # V in [s, hd] layout, all 4 heads at once
                    for sc in range(4):
                        ps = psA.tile([128, 512], F32, tag="a")
                        for dc in range(NDC):
                            _mm(nc, ps[:],
                                xts[dc][:, sc * 128:(sc + 1) * 128],
                                wv_sb[:, dc * HL:(dc + 1) * HL],
                                start=(dc == 0), stop=(dc == NDC - 1))
                        vsb = st1.tile([128, 512], FMM, tag="vsb")
                        nc.scalar.activation(vsb[:], ps[:], AF.Copy)
                        nc.sync.dma_start(
                            out=vs[sb * 512 + sc * 128: sb * 512 + (sc + 1) * 128, :],
                            in_=vsb[:],
                        )

            # ---------------- phase 2: attention ----------------
            with tc.tile_pool(name="ctxp", bufs=1) as ctxp:
                ctxs = [ctxp.tile([128, S], FMM, name=f"ctxT{h}") for h in range(HPC)]
                with (
                    tc.tile_pool(name="qkv2", bufs=2) as qkv2,
                    tc.tile_pool(name="pp", bufs=6) as pp,
                    tc.tile_pool(name="sm", bufs=2) as sm,
                ):
                    for h in range(HPC):
                        qt = qkv2.tile([128, S], FMM, tag="qt")
                        nc.sync.dma_start(out=qt[:], in_=qTs[h])
                        kt = qkv2.tile([128, S], FMM, tag="kt")
                        nc.sync.dma_start(out=kt[:], in_=kTs[h])
                        vt = qkv2.tile([128, NKC, HD], FMM, tag="vt")
                        nc.sync.dma_start(
                            out=vt[:],
                            in_=vs.rearrange("(kc p) c -> p kc c", p=128)[:, :, h * HD:(h + 1) * HD],
                        )

                        for qb in range(NSB):
                            nk = 4 * qb + 4
                            lps = psC.tile([1, 512], F32, tag="c")
                            pv = psB.tile([128, 512], F32, tag="b")
                            for kc in range(nk):
                                j = kc - 4 * qb
                                ncols = 512 if j < 0 else 512 - 128 * j
                                q0 = qb * 512 + 512 - ncols
                                sps = psA.tile([128, 512], F32, tag="a")
                                _mm(nc, sps[:, :ncols],
                                    kt[:, kc * 128:(kc + 1) * 128],
                                    qt[:, q0: (qb + 1) * 512],
                                    start=True, stop=True)
                                pt = pp.tile([128, 512], FMM, tag="pt")
                                nc.scalar.activation(pt[:, :ncols], sps[:, :ncols], AF.Exp)
                                if j >= 0:
                                    nc.vector.tensor_mul(pt[:, :ncols], pt[:, :ncols],
                                                         mask_sb[:, :ncols])
                                _mm(nc, lps[:, 512 - ncols:], ones_col,
                                    pt[:, :ncols],
                                    start=(kc == 0), stop=(kc == nk - 1))
                                _mm(nc, pv[:, 512 - ncols:], vt[:, kc, :],
                                    pt[:, :ncols],
                                    start=(kc == 0), stop=(kc == nk - 1))
                            lsb = sm.tile([1, 512], F32, tag="lsb")
                            nc.scalar.activation(lsb[:], lps[:], AF.Copy)
                            lrec = sm.tile([1, 512], FMM, tag="lrec")
                            with nc.allow_low_precision(reason="fp32r rounding of softmax scale"):
                                nc.vector.reciprocal(lrec[:], lsb[:])
                            reps = psC.tile([128, 512], F32, tag="c")
                            _mm(nc, reps[:], ones_row, lrec[:], start=True, stop=True)
                            repsb = sm.tile([128, 512], F32, tag="repsb")
                            nc.scalar.activation(repsb[:], reps[:], AF.Copy)
                            nc.vector.tensor_mul(ctxs[h][:, qb * 512:(qb + 1) * 512],
                                                 pv[:], repsb[:])

                # ---------------- phase 3: output projection ----------------
                with (
                    tc.tile_pool(name="wop", bufs=1) as wop,
                    tc.tile_pool(name="outp", bufs=3) as outp,
                ):
                    wo_sb = wop.tile([128, HPC * D], FMM, name="wo_sb")
                    for r in range(HPC):
                        nc.sync.dma_start(out=wo_sb[:, r * D:(r + 1) * D],
                                          in_=wo[r * 128:(r + 1) * 128, :])
                    for qc in range(S // 128):
                        osb = outp.tile([128, D], F32, tag="osb")
                        for db in range(D // 512):
                            ops = psA.tile([128, 512], F32, tag="a")
                            for h in range(HPC):
                                _mm(nc, ops[:],
                                    ctxs[h][:, qc * 128:(qc + 1) * 128],
                                    wo_sb[:, h * D + db * 512: h * D + (db + 1) * 512],
                                    start=(h == 0), stop=(h == HPC - 1))
                            nc.scalar.activation(osb[:, db * 512:(db + 1) * 512], ops[:],
                                                 AF.Copy)
                        nc.sync.dma_start(out=out[qc * 128:(qc + 1) * 128, :], in_=osb[:])

    nc.compile()
    return nc


_NC_CACHE = None


def _get_nc():
    global _NC_CACHE
    if _NC_CACHE is None:
        _NC_CACHE = _build()
    return _NC_CACHE


def _host_tables():
    # Replicate reference RoPE tables in float32 arithmetic, transposed.
    inv_freq = np.float32(1.0) / np.power(
        np.float32(ROPE_THETA), np.arange(0, HD, 2).astype(np.float32) / np.float32(HD)
    )
    pos = np.arange(S, dtype=np.float32)
    freqs = pos[:, None] * inv_freq[None, :]
    angles = np.concatenate([freqs, freqs], axis=1)  # [S, HD]
    cos_t = np.ascontiguousarray(np.cos(angles).astype(np.float32).T)  # [HD, S]
    sin_t = np.ascontiguousarray(np.sin(angles).astype(np.float32).T)
    # rotate_half as a left-multiply matrix P: (P q)[2i] = -q[2i+1], [2i+1] = q[2i].
    # matmul computes lhsT.T @ rhs, so feed P.T.
    pmat = np.zeros((HD, HD), dtype=np.float32)
    for i in range(HD // 2):
        pmat[2 * i, 2 * i + 1] = -1.0
        pmat[2 * i + 1, 2 * i] = 1.0
    pmat_t = np.ascontiguousarray(pmat.T)
    mask = (np.arange(128)[:, None] <= np.arange(512)[None, :]).astype(np.float32)
    return cos_t, sin_t, pmat_t, mask


_ONES = np.ones((128, 128), dtype=np.float32)


def kernel(x, Wq, Wk, Wv, Wo):
    x = np.asarray(x, dtype=np.float32)
    Wq = np.asarray(Wq, dtype=np.float32)
    Wk = np.asarray(Wk, dtype=np.float32)
    Wv = np.asarray(Wv, dtype=np.float32)
    Wo = np.asarray(Wo, dtype=np.float32)

    results = _run_device(x, Wq, Wk, Wv, Wo)

    out = np.empty((B, S, D), dtype=np.float32)
    for b in range(B):
        acc = results[b * CORES_PER_BATCH]["out"]
        for i in range(1, CORES_PER_BATCH):
            acc = acc + results[b * CORES_PER_BATCH + i]["out"]
        out[b] = acc
    return out


def _make_in_maps(x, Wq, Wk, Wv, Wo):
    cos_t, sin_t, pmat_t, mask = _host_tables()
    scale = np.float32(1.0 / math.sqrt(HD))
    wq_scaled = (Wq * scale).astype(np.float32)
    xTb = [np.ascontiguousarray(x[b].T) for b in range(B)]
    in_maps = []
    for c in range(N_CORES):
        b = c // CORES_PER_BATCH
        g = c % CORES_PER_BATCH
        hs = slice(g * HL, (g + 1) * HL)
        in_maps.append({
            "xT": xTb[b],
            "wq": np.ascontiguousarray(wq_scaled[:, hs]),
            "wk": np.ascontiguousarray(Wk[:, hs]),
            "wv": np.ascontiguousarray(Wv[:, hs]),
            "wo": np.ascontiguousarray(Wo[hs, :]),
            "cosT": cos_t,
            "sinT": sin_t,
            "pmatT": pmat_t,
            "maskT": mask,
            "onesd": _ONES,
        })
    return in_maps


def _run_device(x, Wq, Wk, Wv, Wo, trace=False):
    nc = _get_nc()
    in_maps = _make_in_maps(x, Wq, Wk, Wv, Wo)
    res = run_bass_kernel_spmd(nc, in_maps, core_ids=list(range(N_CORES)), trace=trace)
    if trace:
        return res
    return res.results


def run_traced(x, Wq, Wk, Wv, Wo):
    """Run with NTFF tracing; returns (full_output, BassKernelResults)."""
    res = _run_device(np.asarray(x, np.float32), np.asarray(Wq, np.float32),
                      np.asarray(Wk, np.float32), np.asarray(Wv, np.float32),
                      np.asarray(Wo, np.float32), trace=True)
    out = np.empty((B, S, D), dtype=np.float32)
    for b in range(B):
        acc = res.results[b * CORES_PER_BATCH]["out"]
        for i in range(1, CORES_PER_BATCH):
            acc = acc + res.results[b * CORES_PER_BATCH + i]["out"]
        out[b] = acc
    return out, res
